# revision 1
# baseline (speedup 1.0000x reference)
"""Self-attention (Base_OC / SAGAN-style) module on Trainium2, 8 NeuronCores.

Problem: x[4, 64, 64, 512]; per batch element b (N = 4096 tokens, C = 512):
  f = x@wf+bf [N,64]; g = x@wg+bg [N,64]; hv = x@wh+bh [N,256]
  s = g @ f^T [N,N]; beta = softmax(s); o = beta @ hv [N,256]
  att = gamma*(o@wo+bo) + x; y = relu(BN([att,x] @ wc + bc))

Sharding: 8 cores = batch(4) x query-row-halves(2). Each core receives x[b]
permuted so its own 2048 query rows come first (attention is permutation-
invariant over keys), computes the pipeline for those rows, returns [2048,512].

The tail is algebraically folded on the host:
  y = relu(o @ W_oc + x @ W_x + B),   W_oc = gamma*(wo @ wc1'),
  W_x = wc1' + wc2',  B = BN-folded bias + gamma*(bo @ wc1')
where wc1'/wc2' are the att/x halves of wc with the BN scale folded in. This
removes the separate o@wo stage, the gamma-residual pass, and drops the
concat matmul contraction from 1024 to 768 effective.

Layout: x is PE-transposed once to xT [c, n]. All dense matmuls run as
float32r (4-byte storage, relaxed-precision PE mode, 4x the fp32 rate; needs
even free sizes + both operands fp32r + dst partition 0). Softmax skips the
max-subtraction (max |logit| ~ 67, exp stays in fp32 range); the softmax
denominator comes from a ones-column appended to hv. s-stage matmuls (K=64)
run pairwise-concurrent in the PE array via tile_position row packing, with
f/g duplicated to both partition halves (other-half f is emitted as [wf|wf]
directly by the projection matmul). Biases ride on VectorE; BN is host-folded.
"""

import numpy as np

import concourse.bacc as bacc
import concourse.mybir as mybir
import concourse.tile as tile
from concourse.bass_utils import run_bass_kernel_spmd

FP = mybir.dt.float32
RR = mybir.dt.float32r
BF = mybir.dt.bfloat16
AF = mybir.ActivationFunctionType
OP = mybir.AluOpType


# View an fp32 AP as float32r for 4x-rate PE matmul (only when N >= 256).
def r32(ap):
    return ap.bitcast(RR)


N_FULL, N_OWN, C, D8, D2 = 4096, 2048, 512, 64, 256
NMT = N_FULL // 128   # 32 key tiles
NCT = C // 128        # 4 channel tiles
NET = D2 // 128       # 2 e tiles
NNB = N_OWN // 512    # 4 query blocks per core
HW2 = 258             # hv width: 256 values | ones col | pad (fp32r needs even)
EPS = 1e-3


def build_program(reps=1):
    nc = bacc.Bacc("TRN2", target_bir_lowering=False, debug=False, num_devices=8)

    xt_d = nc.dram_tensor("xt", [C, N_FULL], BF, kind="ExternalInput").ap()
    wfg_d = nc.dram_tensor("wfg", [C, 256], BF, kind="ExternalInput").ap()
    bfg_d = nc.dram_tensor("bfg", [128, 2], FP, kind="ExternalInput").ap()
    whx_d = nc.dram_tensor("whx", [C, HW2], BF, kind="ExternalInput").ap()
    bhbc_d = nc.dram_tensor("bhbc", [128, HW2], FP, kind="ExternalInput").ap()
    wocx_d = nc.dram_tensor("wocx", [D2, C], BF, kind="ExternalInput").ap()
    wxs_d = nc.dram_tensor("wxs", [C, C], BF, kind="ExternalInput").ap()
    bcol_d = nc.dram_tensor("bcol", [128, NCT], FP, kind="ExternalInput").ap()
    ident_d = nc.dram_tensor("identr", [128, 128], RR, kind="ExternalInput").ap()
    # y is produced transposed [C, N_OWN]; the host untransposes
    y_d = nc.dram_tensor("y", [C, N_OWN], FP, kind="ExternalOutput").ap()

    with tile.TileContext(nc) as tc:
        with (
            tc.tile_pool(name="consts", bufs=1) as cpool,
            tc.tile_pool(name="big", bufs=1) as bigp,
            tc.tile_pool(name="stream", bufs=2) as sp,
            tc.tile_pool(name="exps", bufs=4) as exp_pool,
            tc.tile_pool(name="ysp", bufs=4) as ys_pool,
            tc.tile_pool(name="psB_s", bufs=2, space="PSUM") as ps_pool,
            tc.tile_pool(name="psB_u", bufs=1, space="PSUM") as pu,
        ):
            xT = bigp.tile([128, NCT * N_FULL], BF)   # 64 KB/part
            fT = bigp.tile([128, N_FULL], RR)         # rows 0:64 f, 64:128 dup
            gT = bigp.tile([128, N_OWN], RR)          # rows 64:128 g, 0:64 dup
            hv = bigp.tile([128, NMT * HW2], BF)      # 33 KB
            whx_sb = cpool.tile([128, NCT * HW2], BF)
            wfg_sb = cpool.tile([128, NCT * 256], BF)
            bfg_sb = cpool.tile([128, 2], FP)
            bhbc_sb = cpool.tile([128, HW2], FP)

            def dma_xt(half):
                for t in range(NCT):
                    nc.sync.dma_start(
                        xT[:, t * N_FULL + half * 512: t * N_FULL + (half + 1) * 512],
                        xt_d[t * 128:(t + 1) * 128, half * 512:(half + 1) * 512])

            # critical-path-first DMA order: per-ct wfg/x/whx interleaved so the
            # first fg/hv accumulation chains can start after ~0.5 MB.
            nc.sync.dma_start(bfg_sb, bfg_d)
            for ct in range(NCT):
                nc.sync.dma_start(wfg_sb[:, ct * 256:(ct + 1) * 256],
                                  wfg_d[ct * 128:(ct + 1) * 128, :])
                nc.sync.dma_start(
                    xT[:, ct * N_FULL: ct * N_FULL + 512],
                    xt_d[ct * 128:(ct + 1) * 128, 0:512])
                nc.sync.dma_start(whx_sb[:, ct * HW2:(ct + 1) * HW2],
                                  whx_d[ct * 128:(ct + 1) * 128, :])
            nc.sync.dma_start(bhbc_sb, bhbc_d)

            def emit_hv(mt, phv):
                hp = phv.tile([128, HW2], FP, tag="hv")
                for ct in range(NCT):
                    nc.tensor.matmul(
                        hp,
                        xT[:, ct * N_FULL + mt * 128: ct * N_FULL + (mt + 1) * 128],
                        whx_sb[:, ct * HW2:(ct + 1) * HW2],
                        start=(ct == 0), stop=(ct == NCT - 1))
                # bias (+ones col) via broadcast add, casts to fp32r
                nc.vector.tensor_add(hv[:, mt * HW2:(mt + 1) * HW2], hp, bhbc_sb)

            def emit_fg(ch, pfg):
                cs = slice(ch * 512, (ch + 1) * 512)
                if ch < NNB:
                    # packed [f|g]: out rows 0:64 = f, 64:128 = g
                    fgp = pfg.tile([128, 512], FP, tag="fg")
                    for ct in range(NCT):
                        nc.tensor.matmul(
                            fgp, wfg_sb[:, ct * 256: ct * 256 + 128],
                            xT[:, ct * N_FULL + ch * 512:
                               ct * N_FULL + (ch + 1) * 512],
                            start=(ct == 0), stop=(ct == NCT - 1))
                    nc.vector.tensor_scalar_add(fT[0:D8, cs], fgp[0:D8, :],
                                                bfg_sb[0:D8, 0:1])
                    nc.vector.tensor_scalar_add(gT[D8:128, cs], fgp[D8:128, :],
                                                bfg_sb[D8:128, 0:1])
                    nc.sync.dma_start(fT[D8:128, cs], fT[0:D8, cs])
                    nc.sync.dma_start(gT[0:D8, cs], gT[D8:128, cs])
                else:
                    # other-half keys: [wf|wf] stationary emits f to both
                    # partition halves at once; bias col 1 = [bf;bf]
                    fp_ = pfg.tile([128, 512], FP, tag="fg")
                    for ct in range(NCT):
                        nc.tensor.matmul(
                            fp_, wfg_sb[:, ct * 256 + 128:(ct + 1) * 256],
                            xT[:, ct * N_FULL + ch * 512:
                               ct * N_FULL + (ch + 1) * 512],
                            start=(ct == 0), stop=(ct == NCT - 1))
                    nc.vector.tensor_scalar_add(fT[:, cs], fp_, bfg_sb[:, 1:2])

            def emit_s(nb, mt2):
                # two K=64 s-matmuls concurrent in the PE array + their exps
                nbs = slice(nb * 512, (nb + 1) * 512)
                exs = []
                for half in range(2):
                    mt = 2 * mt2 + half
                    lo, hi = (0, D8) if half == 0 else (D8, 128)
                    sps = ps_pool.tile([128, 512], FP, tag="s")
                    nc.tensor.matmul(
                        sps, r32(fT[lo:hi, mt * 128:(mt + 1) * 128]),
                        r32(gT[lo:hi, nbs]), start=True, stop=True,
                        tile_position=(lo, 0))
                    ex = exp_pool.tile([128, 512], BF, tag="expS")
                    nc.scalar.activation(ex, sps, AF.Exp)
                    exs.append(ex)
                return exs

            def emit_u(mt2, exs, up):
                for half in range(2):
                    mt = 2 * mt2 + half
                    for ns in range(4):
                        nc.tensor.matmul(
                            up[:, ns * 512: ns * 512 + HW2],
                            exs[half][:, ns * 128:(ns + 1) * 128],
                            hv[:, mt * HW2:(mt + 1) * HW2],
                            start=(mt == 0), stop=(mt == NMT - 1))

            def emit_tail(nb, up, pm):
                # normalize -> oT (PE transpose); W_oc path consumes oT in emit_y
                oT = sp.tile([128, NET * 512], BF, tag="oT")
                for ns in range(4):
                    emit_tail_ns(ns, up, oT, pm)
                return oT

            def emit_yT(nb, co, oT, pm, tag="m"):
                # transposed y: out [C-tile co (partitions), 512 queries].
                # Per-partition bias rides the relu activation for free.
                yp = pm.tile([128, 512], FP, tag=tag)
                for ct in range(NCT):
                    nc.tensor.matmul(
                        yp,
                        wxs_sb[:, ct * C + co * 128: ct * C + (co + 1) * 128],
                        xT[:, ct * N_FULL + nb * 512: ct * N_FULL + (nb + 1) * 512],
                        start=(ct == 0), stop=False)
                for et in range(NET):
                    nc.tensor.matmul(
                        yp, wocx_sb[:, et * C + co * 128: et * C + (co + 1) * 128],
                        oT[:, et * 512:(et + 1) * 512],
                        start=False, stop=(et == NET - 1))
                ys = ys_pool.tile([128, 512], FP, tag="ys")
                nc.scalar.activation(ys, yp, AF.Relu, bias=bcol_sb[:, co:co + 1])
                nc.sync.dma_start(
                    y_d[co * 128:(co + 1) * 128, nb * 512:(nb + 1) * 512], ys)

            def emit_norm_ns(ns, up):
                rcp = sp.tile([128, 1], FP, tag="rcp")
                nc.vector.reciprocal(rcp, up[:, ns * 512 + 256: ns * 512 + 257])
                ob = exp_pool.tile([128, D2], RR, tag="ob")
                nc.vector.tensor_scalar_mul(
                    ob, up[:, ns * 512: ns * 512 + 256], rcp)
                return ob

            def emit_trans_ns(ns, ob, oT, pm):
                tp2f = pm.tile([128, 512], FP, tag="m", name="tp2")
                for et in range(NET):
                    tp2 = tp2f[:, et * 128:(et + 1) * 128]
                    nc.tensor.transpose(
                        r32(tp2), ob[:, et * 128:(et + 1) * 128], identr_sb)
                    nc.vector.tensor_copy(
                        oT[:, et * 512 + ns * 128: et * 512 + (ns + 1) * 128], tp2)

            def emit_tail_ns(ns, up, oT, pm):
                emit_trans_ns(ns, emit_norm_ns(ns, up), oT, pm)

            def emit_final(oTp, up, pm):
                # last two query blocks: y(NNB-2) interleaved with the
                # normalize/transpose chains of NNB-1, then y(NNB-1)
                oT = sp.tile([128, NET * 512], BF, tag="oT")
                obs = [emit_norm_ns(ns, up) for ns in range(4)]
                for i in range(4):
                    emit_trans_ns(i, obs[i], oT, pm)
                    emit_yT(NNB - 2, i, oTp, pm)
                for co in range(4):
                    # alternate PSUM rings: ps_pool's s-slots are idle by now
                    emit_yT(NNB - 1, co, oT,
                            pm if co % 2 == 0 else ps_pool,
                            tag="m" if co % 2 == 0 else "s")

            for _rep in range(reps):
                # ---- merged projections + first query block's s/exp/u pipeline ----
                with (
                    tc.tile_pool(name="psA_fg", bufs=1, space="PSUM") as pfg,
                    tc.tile_pool(name="psA_hv", bufs=1, space="PSUM") as phv,
                ):
                    up0 = pu.tile([128, 2048], FP, tag="u")
                    if _rep > 0:
                        dma_xt(0)
                    pend = None   # (mt2, exs) with s/exp emitted, u pending
                    for ch in range(8):
                        if ch < 7:
                            dma_xt(ch + 1)   # prefetch next chunk
                        emit_fg(ch, pfg)
                        emit_hv(4 * ch, phv)
                        emit_hv(4 * ch + 1, phv)
                        exs = emit_s(0, 2 * ch)
                        if pend is not None:
                            emit_u(*pend, up0)
                        pend = (2 * ch, exs)
                        emit_hv(4 * ch + 2, phv)
                        emit_hv(4 * ch + 3, phv)
                        exs = emit_s(0, 2 * ch + 1)
                        emit_u(*pend, up0)
                        pend = (2 * ch + 1, exs)
                        if ch == 1 and _rep == 0:
                            identr_sb = cpool.tile([128, 128], RR)
                            nc.sync.dma_start(identr_sb, ident_d)
                            wocx_sb = cpool.tile([128, NET * C], BF)
                            nc.sync.dma_start(
                                wocx_sb.rearrange("p (t d) -> p t d", t=NET),
                                wocx_d.rearrange("(t p) d -> p t d", p=128))
                        if ch == 3 and _rep == 0:
                            wxs_sb = cpool.tile([128, NCT * C], BF)
                            nc.sync.dma_start(
                                wxs_sb.rearrange("p (t d) -> p t d", t=NCT),
                                wxs_d.rearrange("(t p) d -> p t d", p=128))
                            bcol_sb = cpool.tile([128, NCT], FP)
                            nc.sync.dma_start(bcol_sb, bcol_d)
                    emit_u(*pend, up0)

                # ---- remaining query blocks; s/exp pipelined across nb ----
                with tc.tile_pool(name="psB_m", bufs=2, space="PSUM") as pm:
                    pend2 = [(0, emit_s(1, 0)), (1, emit_s(1, 1))]
                    oT_prev = emit_tail(0, up0, pm)
                    for nb in range(1, NNB):
                        up = pu.tile([128, 2048], FP, tag="u")
                        for k in range(NMT // 2):
                            emit_u(*pend2.pop(0), up)
                            nxt = k + 2
                            if nxt < NMT // 2:
                                pend2.append((nxt, emit_s(nb, nxt)))
                            elif nb + 1 < NNB:
                                m = nxt - NMT // 2
                                pend2.append((m, emit_s(nb + 1, m)))
                        if nb < NNB - 1:
                            # DVE normalize first (frees `up` for the next
                            # block); transposes interleave with y-MMs
                            obs = [emit_norm_ns(ns, up) for ns in range(4)]
                            oT_new = sp.tile([128, NET * 512], BF, tag="oT")
                            for i in range(4):
                                emit_trans_ns(i, obs[i], oT_new, pm)
                                emit_yT(nb - 1, i, oT_prev, pm)
                            oT_prev = oT_new
                    emit_final(oT_prev, up, pm)

    nc.compile()
    return nc


_PROG = None


def _get_prog():
    global _PROG
    if _PROG is None:
        _PROG = build_program()
    return _PROG


def make_in_maps(x, wf, bf, wg, bg, wh, bh, wo, bo, gamma, wc, bc,
                 bn_scale, bn_bias, bn_mean, bn_var):
    import ml_dtypes
    bf16 = ml_dtypes.bfloat16
    f32 = lambda a: np.ascontiguousarray(np.asarray(a, dtype=np.float32))
    b16 = lambda a: np.ascontiguousarray(np.asarray(a, dtype=np.float32).astype(bf16))
    f64 = lambda a: np.asarray(a, np.float64)
    x = f32(x)
    B = x.shape[0]
    xf = x.reshape(B, N_FULL, C)
    gv = float(np.asarray(gamma).ravel()[0])
    sp_ = f64(bn_scale) / np.sqrt(f64(bn_var) + EPS)
    wcs = f64(wc) * sp_[None, :]          # [2C, C] BN-folded concat weight
    wc1, wc2 = wcs[:C], wcs[C:]
    wocx = f32(gv * (f64(wo) @ wc1))      # [C/2, C]
    wxs = f32(wc1 + wc2)                  # [C, C]
    bvec = f32((f64(bc) - f64(bn_mean)) * sp_ + f64(bn_bias)
               + gv * (f64(bo) @ wc1))
    whx = f32(np.concatenate([np.asarray(wh, np.float32),
                              np.zeros((C, 2), np.float32)], axis=1))
    bh_row = np.concatenate([np.asarray(bh, np.float32).ravel(),
                             [1.0, 0.0]]).astype(np.float32)
    wf32, wg32 = f32(wf), f32(wg)
    bf1 = np.asarray(bf, np.float32).ravel()
    bg1 = np.asarray(bg, np.float32).ravel()
    common = dict(
        wfg=b16(np.concatenate([wf32, wg32, wf32, wf32], axis=1)),
        bfg=f32(np.stack([np.concatenate([bf1, bg1]),
                          np.concatenate([bf1, bf1])], axis=1)),
        whx=b16(whx),
        bhbc=np.broadcast_to(bh_row, (128, HW2)).copy(),
        wocx=b16(wocx), wxs=b16(wxs),
        bcol=np.ascontiguousarray(bvec.reshape(NCT, 128).T),
        identr=np.eye(128, dtype=np.float32),
    )
    in_maps = []
    for core in range(8):
        b, h = core // 2, core % 2
        own = xf[b, h * N_OWN:(h + 1) * N_OWN]
        oth = xf[b, (1 - h) * N_OWN:(2 - h) * N_OWN]
        xp = np.concatenate([own, oth], axis=0)
        in_maps.append({"xt": b16(xp.T), **common})
    return in_maps, B


def assemble(results, B):
    out = np.empty((B, N_FULL, C), np.float32)
    for core in range(8):
        b, h = core // 2, core % 2
        out[b, h * N_OWN:(h + 1) * N_OWN] = results[core]["y"].T
    return out.reshape(B, 64, 64, C)


def kernel(**inputs):
    in_maps, B = make_in_maps(**inputs)
    nc = _get_prog()
    res = run_bass_kernel_spmd(nc, in_maps, core_ids=list(range(8)))
    return assemble(res.results, B)

